# revision 29
# baseline (speedup 1.0000x reference)
"""GPT-J attention (B=2, S=2048, D=4096, H=16, HD=256, ROT=64, causal) on 8 TRN2 NeuronCores.

Sharding: DP over batch (2 groups of 4 cores) x TP over heads (4 heads/core).
Each core computes q/k/v projections for its 4 heads (dim-major layouts),
RoPE (host pre-permutes rotary channel pairs so the rotation is two
contiguous 32-partition blocks), causal softmax on transposed scores
(k-tokens on partitions: sums via ones-matmul on PE), attn@V, then a partial
output projection with its Wo row-slice. Four 1MB ReduceScatters per token
tile (bf16, each triggered as soon as its quarter of the partial product is
ready) finish the row-parallel out_proj; DRAM-to-DRAM copies on the gpsimd
DMA queue move the reduced slices into the output parameter, and the host
assembles the 8 cores' slices.

Perf structure:
- All weights host-packed so every weight DMA is a (128, 4096) bf16 tile with
  8KB contiguous per partition (big-line DMA, ~15x fewer descriptors).
- Projection/out-proj passes are split into half-passes (all 32 contraction
  chunks into psum A, then all 32 into psum B) so 3 PSUM accumulator banks
  suffice without stalls.
- Attention is software-pipelined three steps deep: scores(s+1..s+3) are
  issued before the exp(s)-dependent sum/AV matmuls, so the PE doesn't idle
  while the scalar engine computes exp.
- Softmax normalization is fully off-PE: sums broadcast via gpsimd
  partition_broadcast, reciprocal_approx_fast + muls on DVE.
- Diagonal causal blocks are trimmed: score/sum/AV matmuls and exp run only
  on the valid column range; a single 128x128 triangle mask handles the edge.

Compute dtype: bf16 on the TensorEngine (f32 PSUM accumulation).
"""

import numpy as np
import ml_dtypes

import concourse.bass as bass
import concourse.mybir as mybir
import concourse.tile as tile
from concourse import bacc
from concourse.bass_utils import run_bass_kernel_spmd

B, S, D = 2, 2048, 4096
H, HD, ROT = 16, 256, 64
NCORES, DP, TP = 8, 2, 4
NH = H // TP              # 4 local heads
QL = NH * HD              # 1024 local q/k/v dims
T = S                     # local tokens (one batch per DP group)
NT = T // 512             # 4 token tiles of 512
KC = D // 128             # 32 contraction chunks
BF16 = mybir.dt.bfloat16
F32 = mybir.dt.float32
INV_SCALE = 1.0 / 16.0    # 1/sqrt(HD)
GROUPS = [[0, 1, 2, 3], [4, 5, 6, 7]]
NPBF16 = ml_dtypes.bfloat16
Copy = mybir.ActivationFunctionType.Copy
Exp = mybir.ActivationFunctionType.Exp


def build_nc():
    nc = bacc.Bacc("TRN2", target_bir_lowering=False, num_devices=NCORES)
    # x packed [p, c, t]: xp[p, c, t] = x[t, c*128+p]
    xp = nc.declare_dram_parameter("xp", [128, KC, T], BF16, isOutput=False)
    # wq/wk packed: [quarter(head), half, 128, 16*256]
    wq_pk = nc.declare_dram_parameter("wq_pk", [4, 2, 128, 4096], BF16, isOutput=False)
    wk_pk = nc.declare_dram_parameter("wk_pk", [4, 2, 128, 4096], BF16, isOutput=False)
    # wv packed: [vh, quarter, 128, 8*512]
    wv_pk = nc.declare_dram_parameter("wv_pk", [2, 4, 128, 4096], BF16, isOutput=False)
    # wo packed: [ctp_pair, 128, (c2*8+dc)*256]
    wo_pk = nc.declare_dram_parameter("wo_pk", [8, 128, 4096], BF16, isOutput=False)
    cosp = nc.declare_dram_parameter("cosp", [ROT, T], BF16, isOutput=False)
    sinp = nc.declare_dram_parameter("sinp", [ROT, T], BF16, isOutput=False)
    trim = nc.declare_dram_parameter("trim", [128, 128], BF16, isOutput=False)
    # output: [tile, quarter, 256 out-dims, 512 tokens]
    outp = nc.declare_dram_parameter("outp", [NT, 4, 256, 512], BF16, isOutput=True)

    parts = [[nc.dram_tensor(f"part{i}_{q}", [1024, 512], BF16) for q in range(4)]
             for i in range(NT - 1)]
    rsout = [[nc.dram_tensor(f"rs{i}_{q}", [256, 512], BF16) for q in range(4)]
             for i in range(NT - 1)]
    # last tile: two 2MB halves (fewer serialized per-op fixed costs in the tail)
    parts3 = [nc.dram_tensor(f"part3h_{hx}", [2048, 512], BF16) for hx in range(2)]
    rs3 = [nc.dram_tensor(f"rs3h_{hx}", [512, 512], BF16) for hx in range(2)]
    out3 = nc.declare_dram_parameter("out3", [2, 512, 512], BF16, isOutput=True)

    with tile.TileContext(nc) as tc:
        with (
            tc.tile_pool(name="singles", bufs=1) as singles,
            tc.tile_pool(name="xt", bufs=4) as xtp,
            tc.tile_pool(name="wgt", bufs=8) as wgt,
            tc.tile_pool(name="qt", bufs=8) as qtp,
            tc.tile_pool(name="kvres", bufs=1) as kvres,
            tc.tile_pool(name="expp", bufs=6) as expp,
            tc.tile_pool(name="attn", bufs=8) as attnp,
            tc.tile_pool(name="osb", bufs=4) as osbp,
            tc.tile_pool(name="rtmp", bufs=2) as rtmp,
            tc.tile_pool(name="norm", bufs=1) as normp,
            tc.tile_pool(name="acc", bufs=2, space="PSUM") as accp,
            tc.tile_pool(name="ps_s", bufs=3, space="PSUM") as pssp,
            tc.tile_pool(name="avs", bufs=3, space="PSUM") as avsp,
        ):
            # --- startup-critical DMAs first: tile 0's x chunks + first q-pass
            # weights go ahead of cos/sin so the first matmul starts ASAP ---
            pre_xts = []
            for ck in range(4):
                xt_c = xtp.tile([128, 8, 512], BF16, tag="xt", name=f"xt0_{ck}")
                nc.sync.dma_start(out=xt_c, in_=xp[:, 8 * ck:8 * ck + 8, 0:512])
                pre_xts.append(xt_c)
                if ck == 0:
                    pre_wq = []
                    for hf in range(2):
                        w_c = wgt.tile([128, 4096], BF16, tag="wgt", name="w_pre")
                        if hf == 0:
                            nc.sync.dma_start(out=w_c[:, 0:2048], in_=wq_pk[0, 0][:, 0:2048])
                            nc.sync.dma_start(out=w_c[:, 2048:4096], in_=wq_pk[0, 0][:, 2048:4096])
                        else:
                            nc.sync.dma_start(out=w_c, in_=wq_pk[0, hf])
                        pre_wq.append(w_c)
            # --- constants ---
            cos_sb = singles.tile([ROT, T], BF16, name="cos_sb")
            sin_sb = singles.tile([ROT, T], BF16, name="sin_sb")
            nc.sync.dma_start(out=cos_sb, in_=cosp[:, :])
            nc.sync.dma_start(out=sin_sb, in_=sinp[:, :])
            tri_sb = singles.tile([128, 128], BF16, tag="tri", name="tri_sb")
            nc.sync.dma_start(out=tri_sb, in_=trim[:, :])
            ones128 = singles.tile([128, 1], BF16, tag="ones128", name="ones128")
            nc.vector.memset(ones128, 1.0)

            # resident k/v
            kt = [[None] * 8 for _ in range(NT)]   # kt[tt][m]: (128 dq, 512 tok) bf16
            vt = [None] * (NT * 4)                 # vt[jt]: (128 tok, 1024 dv) bf16

            def rope_evac(ps_even, dst, tcols):
                """Evacuate a (128,512) psum tile holding a head's dims 0-127
                (rows 0-31 rot-even, 32-63 rot-odd, 64-127 pass) into dst bf16,
                applying the GPT-J rotation."""
                nc.scalar.activation(out=dst[64:128, :], in_=ps_even[64:128, :],
                                     func=Copy)
                tcos = rtmp.tile([ROT, 512], F32, tag="rtmp", name="tcos")
                nc.vector.tensor_mul(out=tcos, in0=ps_even[0:64, :], in1=cos_sb[:, tcols])
                tsin = rtmp.tile([ROT, 512], F32, tag="rtmp", name="tsin")
                nc.vector.tensor_mul(out=tsin[0:32, :], in0=ps_even[32:64, :], in1=sin_sb[0:32, tcols])
                nc.vector.tensor_mul(out=tsin[32:64, :], in0=ps_even[0:32, :], in1=sin_sb[32:64, tcols])
                nc.vector.tensor_sub(out=dst[0:32, :], in0=tcos[0:32, :], in1=tsin[0:32, :])
                nc.vector.tensor_add(out=dst[32:64, :], in0=tcos[32:64, :], in1=tsin[32:64, :])

            for tt in range(NT):
                tcols = bass.ts(tt, 512)
                # ---------- P(tt): projections ----------
                if tt == 0:
                    xts = pre_xts
                else:
                    xts = []
                    for ck in range(4):
                        xt_c = xtp.tile([128, 8, 512], BF16, tag="xt", name=f"xt{tt}_{ck}")
                        nc.sync.dma_start(out=xt_c, in_=xp[:, 8 * ck:8 * ck + 8, tcols])
                        xts.append(xt_c)

                def xsl(dc):
                    return xts[dc // 8][:, dc % 8, :]

                qtiles = [None] * 8
                for wr, is_q in ((wq_pk, True), (wk_pk, False)):
                    for qr in range(4):
                        if tt == 0 and is_q and qr == 0:
                            chunks = pre_wq
                        else:
                            chunks = []
                            for hf in range(2):
                                w_c = wgt.tile([128, 4096], BF16, tag="wgt", name="w_c")
                                nc.sync.dma_start(out=w_c, in_=wr[qr, hf])
                                chunks.append(w_c)
                        ps0 = accp.tile([128, 512], F32, tag="acc", name="ps0")
                        ps1 = accp.tile([128, 512], F32, tag="acc", name="ps1")
                        for dc in range(KC):
                            wsl = chunks[dc // 16][:, (dc % 16) * 256:(dc % 16) * 256 + 128]
                            nc.tensor.matmul(ps0, wsl, xsl(dc), start=dc == 0, stop=dc == KC - 1)
                        for dc in range(KC):
                            wsl = chunks[dc // 16][:, (dc % 16) * 256 + 128:(dc % 16) * 256 + 256]
                            nc.tensor.matmul(ps1, wsl, xsl(dc), start=dc == 0, stop=dc == KC - 1)
                        if is_q:
                            d0 = qtp.tile([128, 512], BF16, tag="qt", name="qtile")
                            d1 = qtp.tile([128, 512], BF16, tag="qt", name="qtile")
                            qtiles[2 * qr], qtiles[2 * qr + 1] = d0, d1
                        else:
                            d0 = kvres.tile([128, 512], BF16, tag=f"kt{tt}_{2 * qr}", name=f"kt{tt}_{2 * qr}")
                            d1 = kvres.tile([128, 512], BF16, tag=f"kt{tt}_{2 * qr + 1}", name=f"kt{tt}_{2 * qr + 1}")
                            kt[tt][2 * qr], kt[tt][2 * qr + 1] = d0, d1
                        rope_evac(ps0, d0, tcols)
                        nc.scalar.activation(out=d1, in_=ps1, func=Copy)
                # v (token-major): two dv halves x two token-pair passes
                vtl = [kvres.tile([128, QL], BF16, tag=f"vt{tt * 4 + tm}", name=f"vt{tt * 4 + tm}") for tm in range(4)]
                for tm in range(4):
                    vt[tt * 4 + tm] = vtl[tm]
                for vh in range(2):
                    vchunks = []
                    for q4 in range(4):
                        wv_c = wgt.tile([128, 4096], BF16, tag="wgt", name="wv_c")
                        nc.sync.dma_start(out=wv_c, in_=wv_pk[vh, q4])
                        vchunks.append(wv_c)
                    for tmp_i in range(2):
                        pv0 = accp.tile([128, 512], F32, tag="acc", name="pv0")
                        pv1 = accp.tile([128, 512], F32, tag="acc", name="pv1")
                        tm0, tm1 = 2 * tmp_i, 2 * tmp_i + 1
                        for dc in range(KC):
                            wsl = vchunks[dc // 8][:, (dc % 8) * 512:(dc % 8) * 512 + 512]
                            nc.tensor.matmul(pv0, xsl(dc)[:, bass.ts(tm0, 128)], wsl, start=dc == 0, stop=dc == KC - 1)
                        for dc in range(KC):
                            wsl = vchunks[dc // 8][:, (dc % 8) * 512:(dc % 8) * 512 + 512]
                            nc.tensor.matmul(pv1, xsl(dc)[:, bass.ts(tm1, 128)], wsl, start=dc == 0, stop=dc == KC - 1)
                        vcols = bass.ts(vh, 512)
                        nc.scalar.activation(out=vtl[tm0][:, vcols], in_=pv0, func=Copy)
                        nc.scalar.activation(out=vtl[tm1][:, vcols], in_=pv1, func=Copy)

                # ---------- A(tt): attention, one-step software pipeline ----------
                qi = tt
                njt = 4 * qi + 4
                atiles = [None] * 8
                hstate = {}

                def norm_chain(h):
                    sums, av0, av1 = hstate[h]
                    s_sb = normp.tile([1, 512], F32, tag="s_sb", name="s_sb")
                    nc.scalar.activation(out=s_sb, in_=sums, func=Copy)
                    sbc = normp.tile([128, 512], F32, tag="sbc", name="sbc")
                    nc.gpsimd.partition_broadcast(sbc, s_sb)
                    rcp = normp.tile([128, 512], F32, tag="rcp", name="rcp")
                    nc.vector.reciprocal_approx_fast(out=rcp, in_=sbc)
                    a0 = attnp.tile([128, 512], BF16, tag="attn", name="a0")
                    a1 = attnp.tile([128, 512], BF16, tag="attn", name="a1")
                    nc.vector.tensor_mul(out=a0, in0=av0, in1=rcp)
                    nc.vector.tensor_mul(out=a1, in0=av1, in1=rcp)
                    atiles[2 * h], atiles[2 * h + 1] = a0, a1

                def consume(p):
                    h, jt, et, s = p
                    st, sp = jt == 0, jt == njt - 1
                    sums, av0, av1 = hstate[h]
                    nc.tensor.matmul(av0[:, s:], vt[jt][:, bass.ds(h * 256, 128)], et[:, s:], start=st, stop=sp)
                    nc.tensor.matmul(av1[:, s:], vt[jt][:, bass.ds(h * 256 + 128, 128)], et[:, s:], start=st, stop=sp)
                    nc.tensor.matmul(sums[:, s:], ones128, et[:, s:], start=st, stop=sp)
                    if sp:
                        norm_chain(h)

                pending = []
                for h in range(NH):
                    for jt in range(njt):
                        if jt == 0:
                            sums = accp.tile([1, 512], F32, tag="acc", name=f"sums{h}")
                            av0 = avsp.tile([128, 512], F32, tag="avs", name="av0")
                            av1 = avsp.tile([128, 512], F32, tag="avs", name="av1")
                            hstate[h] = (sums, av0, av1)
                        a = jt - 4 * qi
                        s = 128 * a if a > 0 else 0
                        ps = pssp.tile([128, 512], F32, tag="ps_s", name="ps")
                        ktt, kj = jt // 4, jt % 4
                        nc.tensor.matmul(ps[:, s:], kt[ktt][2 * h][:, bass.ts(kj, 128)],
                                         qtiles[2 * h][:, s:], start=True, stop=False)
                        nc.tensor.matmul(ps[:, s:], kt[ktt][2 * h + 1][:, bass.ts(kj, 128)],
                                         qtiles[2 * h + 1][:, s:], start=False, stop=True)
                        et = expp.tile([128, 512], BF16, tag="expp", name="et")
                        nc.scalar.activation(out=et[:, s:], in_=ps[:, s:], func=Exp,
                                             scale=INV_SCALE)
                        if a >= 0:
                            nc.vector.tensor_mul(out=et[:, s:s + 128], in0=et[:, s:s + 128], in1=tri_sb)
                        if len(pending) == 3:
                            consume(pending.pop(0))
                        pending.append((h, jt, et, s))
                for p in pending:
                    consume(p)
                pending = []

                # ---------- O(tt): partial out-projection + split-half RS ----------
                for cp in range(8):
                    wo_c = wgt.tile([128, 4096], BF16, tag="wgt", name="wo_c")
                    nc.sync.dma_start(out=wo_c, in_=wo_pk[cp])
                    for c2 in range(2):
                        ctp = 2 * cp + c2
                        po0 = accp.tile([128, 512], F32, tag="acc", name="po0")
                        po1 = accp.tile([128, 512], F32, tag="acc", name="po1")
                        for dc in range(8):
                            wsl = wo_c[:, (c2 * 8 + dc) * 256:(c2 * 8 + dc) * 256 + 128]
                            nc.tensor.matmul(po0, wsl, atiles[dc], start=dc == 0, stop=dc == 7)
                        for dc in range(8):
                            wsl = wo_c[:, (c2 * 8 + dc) * 256 + 128:(c2 * 8 + dc) * 256 + 256]
                            nc.tensor.matmul(po1, wsl, atiles[dc], start=dc == 0, stop=dc == 7)
                        for half, po in ((0, po0), (1, po1)):
                            o_sb = osbp.tile([128, 512], BF16, tag="osb", name="o_sb")
                            nc.scalar.activation(out=o_sb, in_=po, func=Copy)
                            if tt < NT - 1:
                                dst = parts[tt][ctp // 4][bass.ds((ctp % 4) * 256 + half * 128, 128), :]
                            else:
                                dst = parts3[ctp // 8][bass.ds((ctp % 8) * 256 + half * 128, 128), :]
                            nc.sync.dma_start(out=dst, in_=o_sb)
                    if tt < NT - 1 and cp % 2 == 1:
                        q = cp // 2
                        nc.gpsimd.collective_compute(
                            "ReduceScatter", mybir.AluOpType.add,
                            replica_groups=GROUPS,
                            ins=[parts[tt][q][:]], outs=[rsout[tt][q][:]],
                        )
                        nc.gpsimd.dma_start(out=outp[tt, q], in_=rsout[tt][q][:])
                    if tt == NT - 1 and cp % 4 == 3:
                        hx = cp // 4
                        nc.gpsimd.collective_compute(
                            "ReduceScatter", mybir.AluOpType.add,
                            replica_groups=GROUPS,
                            ins=[parts3[hx][:]], outs=[rs3[hx][:]],
                        )
                        nc.gpsimd.dma_start(out=out3[hx], in_=rs3[hx][:])

    nc.compile()
    return nc


_ROT_PERM = np.concatenate([np.arange(0, ROT, 2), np.arange(1, ROT, 2), np.arange(ROT, HD)])


def make_in_maps(hidden_states, sin, cos, Wq, Wk, Wv, Wo):
    hidden_states = np.asarray(hidden_states, dtype=np.float32)
    sin = np.asarray(sin, dtype=np.float32)
    cos = np.asarray(cos, dtype=np.float32)
    Wq, Wk, Wv, Wo = (np.asarray(w, dtype=np.float32) for w in (Wq, Wk, Wv, Wo))

    ce = cos[0, :, 0, 0::2].T  # (32, S)
    co = cos[0, :, 0, 1::2].T
    se = sin[0, :, 0, 0::2].T
    so = sin[0, :, 0, 1::2].T
    cosp = np.ascontiguousarray(np.concatenate([ce, co], axis=0)).astype(NPBF16)  # (64, S)
    sinp = np.ascontiguousarray(np.concatenate([se, so], axis=0)).astype(NPBF16)

    j = np.arange(128)[:, None]
    i = np.arange(128)[None, :]
    trim = (j <= i).astype(NPBF16)  # (128,128) upper triangle

    def pack_qk(wT):
        # (D, QL) -> (4 quarter, 2 half, 128, 16*256)
        w5 = wT.reshape(2, 16, 128, 4, 256)
        return np.ascontiguousarray(w5.transpose(3, 0, 2, 1, 4).reshape(4, 2, 128, 4096))

    def pack_v(wvT):
        # (D, QL) -> (2 vh, 4 quarter, 128, 8*512)
        w5 = wvT.reshape(4, 8, 128, 2, 512)
        return np.ascontiguousarray(w5.transpose(3, 0, 2, 1, 4).reshape(2, 4, 128, 4096))

    def pack_wo(woT):
        # (QL, D) -> (8 cp, 128, (c2*8+dc)*256)
        w5 = woT.reshape(8, 128, 8, 2, 256)
        return np.ascontiguousarray(w5.transpose(2, 1, 3, 0, 4).reshape(8, 128, 4096))

    in_maps = []
    for c in range(NCORES):
        dp, tp = divmod(c, TP)
        xT = np.ascontiguousarray(hidden_states[dp].T).astype(NPBF16)  # (D, T)
        xp = np.ascontiguousarray(xT.reshape(KC, 128, T).transpose(1, 0, 2))
        rows = np.arange(QL * tp, QL * (tp + 1))
        perm_rows = np.concatenate([QL * tp + 256 * h + _ROT_PERM for h in range(NH)])
        wqT = np.ascontiguousarray(Wq[perm_rows].T).astype(NPBF16)  # (D, QL)
        wkT = np.ascontiguousarray(Wk[perm_rows].T).astype(NPBF16)
        wvT = np.ascontiguousarray(Wv[rows].T).astype(NPBF16)
        woT = Wo[:, rows].T.astype(NPBF16)                          # (QL, D)
        in_maps.append({
            "xp": xp,
            "wq_pk": pack_qk(wqT),
            "wk_pk": pack_qk(wkT),
            "wv_pk": pack_v(wvT),
            "wo_pk": pack_wo(woT),
            "cosp": cosp,
            "sinp": sinp,
            "trim": trim,
        })
    return in_maps


def assemble_output(results):
    out = np.empty((B, S, D), dtype=np.float32)
    for dp in range(DP):
        for r in range(TP):
            res = results[dp * TP + r]
            o = np.asarray(res["outp"]).astype(np.float32).reshape(NT, 4, 256, 512)
            for tt in range(NT - 1):
                tok = slice(tt * 512, (tt + 1) * 512)
                for q in range(4):
                    base = 1024 * q + 256 * r
                    out[dp, tok, base:base + 256] = o[tt, q].T
            o3 = np.asarray(res["out3"]).astype(np.float32).reshape(2, 512, 512)
            tok = slice((NT - 1) * 512, NT * 512)
            for hx in range(2):
                base = 2048 * hx + 512 * r
                out[dp, tok, base:base + 512] = o3[hx].T
    return out


_NC_CACHE = None


def _get_nc():
    global _NC_CACHE
    if _NC_CACHE is None:
        _NC_CACHE = build_nc()
    return _NC_CACHE


def run(trace=False, **inputs):
    in_maps = make_in_maps(**inputs)
    nc = _get_nc()
    r = run_bass_kernel_spmd(nc, in_maps, core_ids=list(range(NCORES)), trace=trace)
    return assemble_output(r.results), r


def kernel(**inputs):
    out, _ = run(trace=False, **inputs)
    return out
